# revision 18
# baseline (speedup 1.0000x reference)
"""AttentionBlock (GroupNorm + 8-head self-attention + proj + residual) on 8 TRN2 cores.

Sharding: data-parallel over batch (8 images -> 8 cores). Each core runs the full
block for one [512, 1024] image.

v2 design (vs the bf16 baseline):
  - All four matmul groups (QKV, scores, AV for heads 0-3, proj) run fp8(e4m3)
    DoubleRow (0.5 cyc/row). fp8 quantization is safe: the attention path is
    ~25x diluted vs the fp32 residual.
  - Scores per head: contraction d=64 at partition base (h%2)*64; DoubleRow
    plane 1 of lhsT points at a zeroed slot of the K tile, and the rhs reuses
    the same Q slab via a stride-0 plane (0 * Q = 0).
  - Softmax exp (64 [128,1024] psum tiles) is the elementwise bottleneck and
    PSUM is reachable only by ACT and DVE (GPSIMD/Pool is SBUF-only), so exp
    splits: ACT runs true exp (heads 0-3 -> fp8; heads 4-5 + part of 6 -> bf16
    written through a u16 bitcast), DVE runs the Schraudolph bit-hack
    (u16 = round(s*EA+EB) == bf16 bits of exp(s/8)) for head 7 + rest of 6.
    AV for heads 4-7 consumes the u16 tiles bitcast to bf16 (plain matmul).
  - AV stationary is [v | ones] (even heads) / [ones | v] (odd), so psum rows
    carry att on the head's own 64 partitions and 64 replicated softmax
    denominators Z on the other half: normalize = reciprocal_approx_fast +
    one psum-eviction multiply; no partition broadcast, no DMA staging.
  - qkv_b's v-part is folded into proj_b on the host (proj_w @ vb), removing
    the V bias tensor; V evictions are plain ACT Identity converts.
  - GroupNorm stats run on a bf16 copy of x (fast DMA); fp32 x streams in
    behind the weights and is only read by the final residual.
"""
import sys

sys.path.insert(0, "/opt/trn_rl_repo")

import numpy as np
import ml_dtypes

import concourse.bass as bass
import concourse.bacc as bacc
import concourse.tile as tile
from concourse import mybir
from concourse.bass_utils import run_bass_kernel_spmd

F32 = mybir.dt.float32
I32 = mybir.dt.int32
BF16 = mybir.dt.bfloat16
F8 = mybir.dt.float8e4
U16 = mybir.dt.uint16
AF = mybir.ActivationFunctionType
OP = mybir.AluOpType
DR = mybir.MatmulPerfMode.DoubleRow
NPBF16 = ml_dtypes.bfloat16
NPF8 = ml_dtypes.float8_e4m3

P = 128
S = 1024
CT = 4
HEADS = 8
D = 64
N_CORES = 8
EPS = 1e-5

# Schraudolph exp constants, folded with the softmax scale (1/8) and the
# >>16 that lands the bf16 bit pattern in a u16.
EA = (2.0**23 / float(np.log(2.0))) * 0.125 / 65536.0
EB = (127 * 2**23 - 486411) / 65536.0
# magic-constant fast reciprocal (bitcast(MAGIC - bits(z)) ~ 1/z, ~5% max err)
MAGIC = 0x7EF31200

F8_HEADS = (0, 1, 2, 3)    # ACT exp -> fp8 pt, DoubleRow AV
B16_HEADS = (4, 5, 6, 7)   # u16 pt (bf16 bits), plain bf16 AV
# exp engine per (head, chunk): ACT h0-5 + h6 c0-3; DVE-hack h6 c4-7, h7.
ACT_EXP = {(h, c) for h in range(6) for c in range(8)} | {(6, c) for c in range(4)}


def _emit(nc, tc, ctx):
    io = {
        "xb": nc.dram_tensor("xb", [512, S], BF16, kind="ExternalInput"),
        "x": nc.dram_tensor("x", [512, S], F32, kind="ExternalInput"),
        "wqkv": nc.dram_tensor("wqkv", [P, 2, 2, 1536], F8, kind="ExternalInput"),
        "wproj": nc.dram_tensor("wproj", [P, 2, 2, 512], F8, kind="ExternalInput"),
        "gnw": nc.dram_tensor("gnw", [P, CT], F32, kind="ExternalInput"),
        "gnb": nc.dram_tensor("gnb", [P, CT], F32, kind="ExternalInput"),
        "qkvb": nc.dram_tensor("qkvb", [P, 8], F32, kind="ExternalInput"),
        "projb": nc.dram_tensor("projb", [P, CT], F32, kind="ExternalInput"),
        "sel": nc.dram_tensor("sel", [P, P], BF16, kind="ExternalInput"),
        "selt": nc.dram_tensor("selt", [P, P], BF16, kind="ExternalInput"),
        "out": nc.dram_tensor("out", [512, S], F32, kind="ExternalOutput"),
    }
    _emit_io(nc, tc, ctx, io)


def _emit_io(nc, tc, ctx, io):
    consts = ctx.enter_context(tc.tile_pool(name="consts", bufs=1))
    big = ctx.enter_context(tc.tile_pool(name="big", bufs=1))
    small = ctx.enter_context(tc.tile_pool(name="small", bufs=2))
    ptf8p = ctx.enter_context(tc.tile_pool(name="ptf8", bufs=4))
    ptu16p = ctx.enter_context(tc.tile_pool(name="ptu16", bufs=3))
    zbp = ctx.enter_context(tc.tile_pool(name="zbp", bufs=2))
    outp = ctx.enter_context(tc.tile_pool(name="outp", bufs=3))
    ps = ctx.enter_context(tc.tile_pool(name="ps", bufs=4, space="PSUM"))

    # ---- input DMAs (bf16 x first: GroupNorm needs it immediately) ----
    xbv = io["xb"][:, :].rearrange("(j p) s -> p j s", p=P)
    xb = big.tile([P, CT, S], BF16)
    for j in range(CT):
        for hf in range(4):
            nc.sync.dma_start(out=xb[:, j, hf * 256:(hf + 1) * 256],
                              in_=xbv[:, j, hf * 256:(hf + 1) * 256])
    wqkv = consts.tile([P, 2, 2, 1536], F8)
    for pair in range(2):
        for pl in range(2):
            nc.sync.dma_start(out=wqkv[:, pair, pl, :], in_=io["wqkv"][:, pair, pl, :])
    gnw = consts.tile([P, CT], F32)
    nc.sync.dma_start(out=gnw, in_=io["gnw"][:, :])
    gnb = consts.tile([P, CT], F32)
    nc.sync.dma_start(out=gnb, in_=io["gnb"][:, :])
    sel = consts.tile([P, P], BF16)
    nc.sync.dma_start(out=sel, in_=io["sel"][:, :])
    selt = consts.tile([P, P], BF16)
    nc.sync.dma_start(out=selt, in_=io["selt"][:, :])
    qkvb = consts.tile([P, 8], F32)
    nc.sync.dma_start(out=qkvb, in_=io["qkvb"][:, :])
    projb = consts.tile([P, CT], F32)
    nc.sync.dma_start(out=projb, in_=io["projb"][:, :])
    x_all = big.tile([P, CT, S], F32)
    xv = io["x"][:, :].rearrange("(j p) s -> p j s", p=P)
    for j in range(CT):
        nc.sync.dma_start(out=x_all[:, j, :], in_=xv[:, j, :])
    wproj = consts.tile([P, 2, 2, 512], F8)
    nc.sync.dma_start(out=wproj, in_=io["wproj"][:, :, :, :])

    eps_t = consts.tile([P, 1], F32)
    nc.vector.memset(eps_t, EPS)
    zeros8 = consts.tile([P, 8], F32)
    nc.vector.memset(zeros8, 0.0)

    h = big.tile([P, CT, S], F8)
    qf8 = big.tile([P, CT, S], F8)
    # K storage: [p, head, chunk, 128]. Head h occupies partitions
    # (h%2)*64..+64; the other 64 rows stay zero so the scores matmul can
    # contract over the full 128 partitions of the stacked Q pair tile.
    kf8 = big.tile([P, HEADS, 8, P], F8)
    for hh in range(HEADS):
        zb_base = 0 if hh % 2 == 1 else D
        nc.gpsimd.memset(kf8[zb_base:zb_base + D, hh, :, :], 0.0)
    # V stationary [v | ones] (even heads) / [ones | v] (odd): psum rows get
    # att on the head's own half and 64 replicated Z rows on the other half.
    vtf8 = big.tile([P, 8, len(F8_HEADS), P], F8)
    vtb16 = big.tile([P, 8, len(B16_HEADS), P], BF16)
    for i, hh in enumerate(F8_HEADS):
        off = D if hh % 2 == 0 else 0
        nc.gpsimd.memset(vtf8[:, :, i, off:off + D], 1.0)
    for i, hh in enumerate(B16_HEADS):
        off = D if hh % 2 == 0 else 0
        nc.gpsimd.memset(vtb16[:, :, i, off:off + D], 1.0)
    att = big.tile([P, CT, S], F8)

    # ---- GroupNorm, per chunk so h chunks release as xb arrives ----
    scb = small.tile([P, CT, 2], F32)
    for j in range(CT):
        stats = small.tile([P, 2, 6], F32, tag="st", name=f"st{j}")
        for sg in range(2):
            nc.vector.bn_stats(out=stats[:, sg, :], in_=xb[:, j, sg * 512:(sg + 1) * 512])
        mvj = small.tile([P, 2], F32, tag="mv", name=f"mv{j}")
        nc.vector.bn_aggr(out=mvj, in_=stats[:, :, :])
        s2 = small.tile([P, 2], F32, tag="s2", name=f"s2{j}")
        nc.vector.tensor_copy(out=s2[:, 0:1], in_=mvj[:, 0:1])
        nc.vector.scalar_tensor_tensor(out=s2[:, 1:2], in0=mvj[:, 0:1],
                                       scalar=mvj[:, 0:1], in1=mvj[:, 1:2],
                                       op0=OP.mult, op1=OP.add)
        s2r = small.tile([P, 2], BF16, tag="s2r", name=f"s2r{j}")
        nc.vector.tensor_copy(out=s2r, in_=s2)
        psg = ps.tile([P, 2], F32, tag="mm", name=f"psg{j}")
        nc.tensor.matmul(psg[:, :], lhsT=sel[:, :], rhs=s2r[:, :], start=True, stop=True)
        tmv = small.tile([P, 2], F32, tag="tmv", name=f"tmv{j}")
        nc.vector.tensor_scalar_mul(out=tmv[0:8, :], in0=psg[0:8, :], scalar1=1.0 / 16.0)
        var_t = small.tile([P, 1], F32, tag="vt", name=f"vt{j}")
        nc.vector.tensor_mul(out=var_t[0:8, :], in0=tmv[0:8, 0:1], in1=tmv[0:8, 0:1])
        nc.vector.tensor_sub(out=var_t[0:8, :], in0=tmv[0:8, 1:2], in1=var_t[0:8, :])
        nc.scalar.activation(out=var_t[0:8, :], in_=var_t[0:8, :], func=AF.Sqrt,
                             bias=eps_t[0:8, :], scale=1.0)
        ab = small.tile([P, 2], BF16, tag="ab", name=f"ab{j}")
        nc.vector.tensor_copy(out=ab, in_=zeros8[:, 0:2])
        a_t = small.tile([P, 1], F32, tag="at", name=f"at{j}")
        nc.vector.reciprocal(out=a_t[0:8, :], in_=var_t[0:8, :])
        nc.vector.tensor_copy(out=ab[0:8, 0:1], in_=a_t[0:8, :])
        nc.vector.tensor_scalar(out=ab[0:8, 1:2], in0=tmv[0:8, 0:1],
                                scalar1=a_t[0:8, :], scalar2=-1.0, op0=OP.mult, op1=OP.mult)
        pab = ps.tile([P, 2], F32, tag="mm", name=f"pab{j}")
        nc.tensor.matmul(pab[:, :], lhsT=selt[:, :], rhs=ab[:, :], start=True, stop=True)
        nc.vector.tensor_mul(out=scb[:, j, 0:1], in0=pab[:, 0:1], in1=gnw[:, j:j + 1])
        nc.vector.tensor_mul(out=scb[:, j, 1:2], in0=pab[:, 1:2], in1=gnw[:, j:j + 1])
        nc.vector.tensor_add(out=scb[:, j, 1:2], in0=scb[:, j, 1:2], in1=gnb[:, j:j + 1])
        # GN apply: h = xb * a + b -> fp8. Chunks 0-1 on ACT (fast QKV
        # start), 2-3 on Pool (the only psum-free engine).
        if j < 2:
            nc.scalar.activation(out=h[:, j, :], in_=xb[:, j, :], func=AF.Identity,
                                 bias=scb[:, j, 1:2], scale=scb[:, j, 0:1])
        else:
            nc.gpsimd.tensor_scalar(out=h[:, j, :], in0=xb[:, j, :],
                                    scalar1=scb[:, j, 0:1], scalar2=scb[:, j, 1:2],
                                    op0=OP.mult, op1=OP.add)

    # ---- QKV emitters ----
    def emit_k(ko):
        pk = ps.tile([P, S], F32, tag="mm", name=f"pk{ko}")
        oc = (4 + ko) * P
        for pair in range(2):
            for nh in range(2):
                nc.tensor.matmul(
                    pk[:, nh * 512:(nh + 1) * 512],
                    lhsT=wqkv[:, pair, :, oc:oc + P],
                    rhs=h[:, 2 * pair:2 * pair + 2, nh * 512:(nh + 1) * 512],
                    start=(pair == 0), stop=(pair == 1), perf_mode=DR,
                )
        h_even, h_odd = 2 * ko, 2 * ko + 1
        if ko in (0, 3):
            nc.vector.tensor_scalar(
                out=kf8[0:D, h_even, :, :],
                in0=pk[0:D, :].rearrange("p (c n) -> p c n", c=8),
                scalar1=qkvb[0:D, 4 + ko:5 + ko], scalar2=None, op0=OP.add)
            nc.vector.tensor_scalar(
                out=kf8[D:P, h_odd, :, :],
                in0=pk[D:P, :].rearrange("p (c n) -> p c n", c=8),
                scalar1=qkvb[D:P, 4 + ko:5 + ko], scalar2=None, op0=OP.add)
        else:
            nc.scalar.activation(
                out=kf8[0:D, h_even, :, :],
                in_=pk[0:D, :].rearrange("p (c n) -> p c n", c=8),
                func=AF.Identity, bias=qkvb[0:D, 4 + ko:5 + ko], scale=1.0)
            nc.scalar.activation(
                out=kf8[D:P, h_odd, :, :],
                in_=pk[D:P, :].rearrange("p (c n) -> p c n", c=8),
                func=AF.Identity, bias=qkvb[D:P, 4 + ko:5 + ko], scale=1.0)

    def emit_q(pc):
        pq = ps.tile([P, S], F32, tag="mm", name=f"pq{pc}")
        for pair in range(2):
            for nh in range(2):
                nc.tensor.matmul(
                    pq[:, nh * 512:(nh + 1) * 512],
                    lhsT=wqkv[:, pair, :, pc * P:(pc + 1) * P],
                    rhs=h[:, 2 * pair:2 * pair + 2, nh * 512:(nh + 1) * 512],
                    start=(pair == 0), stop=(pair == 1), perf_mode=DR,
                )
        nc.scalar.activation(out=qf8[:, pc, :], in_=pq[:, :], func=AF.Identity,
                             bias=qkvb[:, pc:pc + 1], scale=1.0)

    def emit_v(si):
        pv = ps.tile([P, 512], F32, tag="mm", name=f"pv{si}")
        for pair in range(2):
            nc.tensor.matmul(
                pv[:, :],
                lhsT=h[:, 2 * pair:2 * pair + 2, si * P:(si + 1) * P],
                rhs=wqkv[:, pair, :, 1024:1536],
                start=(pair == 0), stop=(pair == 1), perf_mode=DR,
            )
        src = pv[:, :].rearrange("p (hh d) -> p hh d", hh=8)
        # per (dtype, parity) group; stride-2 head slices keep APs regular
        for dst, heads, par in ((vtf8, F8_HEADS, 0), (vtf8, F8_HEADS, 1),
                                (vtb16, B16_HEADS, 0), (vtb16, B16_HEADS, 1)):
            idxs = [i for i, hh in enumerate(heads) if hh % 2 == par]
            gh = [heads[i] for i in idxs]
            off = 0 if par == 0 else D
            st = (idxs[1] - idxs[0]) if len(idxs) > 1 else 1
            d0 = dst[:, si, idxs[0]:idxs[-1] + 1:st, off:off + D]
            s0 = src[:, gh[0]:gh[-1] + 1:2, :] if len(gh) > 1 else src[:, gh[0]:gh[0] + 1, :]
            nc.vector.tensor_copy(out=d0, in_=s0)

    def emit_score_chunk(hh, c, pt):
        psc = ps.tile([P, S], F32, tag="mm", name=f"sc{hh}_{c}")
        for ah in range(2):
            nc.tensor.matmul(
                psc[:, ah * 512:(ah + 1) * 512],
                lhsT=kf8[:, hh, c, :],
                rhs=qf8[:, hh // 2, ah * 512:(ah + 1) * 512],
                start=True, stop=True,
            )
        if hh in F8_HEADS:
            nc.scalar.activation(out=pt[:, c, :], in_=psc[:, :], func=AF.Exp, scale=0.125)
        elif (hh, c) in ACT_EXP:
            nc.scalar.activation(out=pt[:, c, :].bitcast(BF16), in_=psc[:, :],
                                 func=AF.Exp, scale=0.125)
        else:
            nc.vector.tensor_scalar(out=pt[:, c, :], in0=psc[:, :],
                                    scalar1=EA, scalar2=EB, op0=OP.mult, op1=OP.add)

    def new_pt(hh):
        if hh in F8_HEADS:
            return ptf8p.tile([P, 8, S], F8, tag="pt", name=f"ptf8_{hh}")
        return ptu16p.tile([P, 8, S], U16, tag="pt", name=f"ptu16_{hh}")

    def av_steps(hh):
        return 4 if hh in F8_HEADS else 8

    def emit_av_mms(hh, pt, pav, step):
        last = av_steps(hh) - 1
        if hh in F8_HEADS:
            vt = vtf8[:, :, F8_HEADS.index(hh), :]
            for ah in range(2):
                nc.tensor.matmul(
                    pav[:, ah * 512:(ah + 1) * 512],
                    lhsT=vt[:, 2 * step:2 * step + 2, :],
                    rhs=pt[:, 2 * step:2 * step + 2, ah * 512:(ah + 1) * 512],
                    start=(step == 0), stop=(step == last), perf_mode=DR,
                    skip_group_check=True,
                )
        else:
            vt = vtb16[:, :, B16_HEADS.index(hh), :]
            for ah in range(2):
                nc.tensor.matmul(
                    pav[:, ah * 512:(ah + 1) * 512],
                    lhsT=vt[:, step, :],
                    rhs=pt[:, step, ah * 512:(ah + 1) * 512].bitcast(BF16),
                    start=(step == 0), stop=(step == last),
                    skip_group_check=True,
                )

    def emit_av(hh, pt, pav=None):
        if pav is None:
            pav = ps.tile([P, S], F32, tag="mm", name=f"av{hh}")
            for step in range(av_steps(hh)):
                emit_av_mms(hh, pt, pav, step)
        # normalize; even head: att rows 0:64, Z rows 64:128 (odd: flipped).
        # 1/Z via the magic-constant int trick: DVE microcoded reciprocal ops
        # read garbage from PSUM on real HW, tensor_scalar int ops don't.
        emit_av_norm(hh, pav)

    def emit_av_norm(hh, pav):
        zbinv = zbp.tile([P, S], I32, tag="zb", name=f"zb{hh}")
        if hh % 2 == 0:
            nc.vector.tensor_scalar(out=zbinv[D:P, :], in0=pav[D:P, :].bitcast(I32),
                                    scalar1=-1, scalar2=MAGIC, op0=OP.mult, op1=OP.add)
            nc.vector.tensor_mul(out=att[0:D, hh // 2, :], in0=pav[0:D, :],
                                 in1=zbinv[D:P, :].bitcast(F32))
        else:
            nc.vector.tensor_scalar(out=zbinv[0:D, :], in0=pav[0:D, :].bitcast(I32),
                                    scalar1=-1, scalar2=MAGIC, op0=OP.mult, op1=OP.add)
            nc.vector.tensor_mul(out=att[D:P, hh // 2, :], in0=pav[D:P, :],
                                 in1=zbinv[0:D, :].bitcast(F32))

    # ---- proj + bias + residual ----
    out_view = io["out"][:, :].rearrange("(j p) s -> p j s", p=P)

    def proj_mms(pp, oi, pair):
        for sh in range(2):
            nc.tensor.matmul(
                pp[:, sh * 512:(sh + 1) * 512],
                lhsT=wproj[:, pair, :, oi * P:(oi + 1) * P],
                rhs=att[:, 2 * pair:2 * pair + 2, sh * 512:(sh + 1) * 512],
                start=(pair == 0), stop=(pair == 1), perf_mode=DR,
                skip_group_check=True,
            )

    def proj_evict(pp, oi):
        ot = outp.tile([P, S], F32, tag="o")
        nc.vector.scalar_tensor_tensor(out=ot, in0=pp[:, :], scalar=projb[:, oi:oi + 1],
                                       in1=x_all[:, oi, :], op0=OP.add, op1=OP.add)
        nc.sync.dma_start(out=out_view[:, oi, :], in_=ot)


    # ---- emission schedule ----
    # PE: all K/Q | sc0+sc6 | sc1+sc7 | V | sc2 | av0 av6 | sc3 | av1 av7 |
    #     sc4 | av2 | sc5 | av3 av4 av5 | proj.  ACT exp streams h0..h5;
    # DVE hacks h6, h7 early, h5 tail; Pool = memsets + GN applies.
    pts = {hh: None for hh in range(8)}
    emit_k(0)
    emit_q(0)
    pts[0] = new_pt(0)
    emit_score_chunk(0, 0, pts[0])
    emit_k(3)
    emit_score_chunk(0, 1, pts[0])
    emit_q(3)
    emit_score_chunk(0, 2, pts[0])
    emit_k(1)
    emit_score_chunk(0, 3, pts[0])
    emit_q(1)
    emit_score_chunk(0, 4, pts[0])
    emit_k(2)
    emit_score_chunk(0, 5, pts[0])
    emit_q(2)
    emit_score_chunk(0, 6, pts[0])
    emit_score_chunk(0, 7, pts[0])
    pts[1] = new_pt(1)
    pts[6] = new_pt(6)
    for c in range(8):
        emit_score_chunk(1, c, pts[1])
        emit_score_chunk(6, c, pts[6])
    pts[2] = new_pt(2)
    pts[7] = new_pt(7)
    for c in range(8):
        emit_score_chunk(2, c, pts[2])
        emit_score_chunk(7, c, pts[7])
    for si in range(8):
        emit_v(si)
    pts[3] = new_pt(3)
    for c in range(8):
        emit_score_chunk(3, c, pts[3])
    emit_av(0, pts[0])
    emit_av(6, pts[6])
    pts[4] = new_pt(4)
    for c in range(8):
        emit_score_chunk(4, c, pts[4])
    emit_av(1, pts[1])
    emit_av(7, pts[7])
    pts[5] = new_pt(5)
    for c in range(8):
        emit_score_chunk(5, c, pts[5])
    emit_av(2, pts[2])
    emit_av(3, pts[3])
    # att pair 0 (heads 0-3) is final: start proj partials for oi 0-1 while
    # the last two AVs run (2 psum bufs each side)
    pp0 = ps.tile([P, S], F32, tag="mm", name="pp0")
    proj_mms(pp0, 0, 0)
    pp1 = ps.tile([P, S], F32, tag="mm", name="pp1")
    proj_mms(pp1, 1, 0)
    emit_av(4, pts[4])
    emit_av(5, pts[5])
    proj_mms(pp0, 0, 1)
    proj_evict(pp0, 0)
    proj_mms(pp1, 1, 1)
    proj_evict(pp1, 1)

    for oi in (2, 3):
        pp = ps.tile([P, S], F32, tag="mm", name=f"pp{oi}")
        proj_mms(pp, oi, 0)
        proj_mms(pp, oi, 1)
        proj_evict(pp, oi)


_NC_CACHE = None


def _build():
    global _NC_CACHE
    if _NC_CACHE is None:
        from contextlib import ExitStack

        nc = bacc.Bacc()
        with tile.TileContext(nc) as tc:
            with ExitStack() as ctx:
                _emit(nc, tc, ctx)
        nc.finalize()
        _NC_CACHE = nc
    return _NC_CACHE


def _prep_inputs(inputs):
    x = np.ascontiguousarray(np.asarray(inputs["x"], dtype=np.float32))  # [8,512,32,32]
    gn_w = np.asarray(inputs["gn_w"], dtype=np.float32)
    gn_b = np.asarray(inputs["gn_b"], dtype=np.float32)
    qkv_w = np.asarray(inputs["qkv_w"], dtype=np.float32)
    qkv_b = np.asarray(inputs["qkv_b"], dtype=np.float32)
    proj_w = np.asarray(inputs["proj_w"], dtype=np.float32)
    proj_b = np.asarray(inputs["proj_b"], dtype=np.float32)

    # [p, pair, plane, o] with c = (2*pair+plane)*128 + p
    wqkv_p = np.ascontiguousarray(
        qkv_w.T.reshape(2, 2, P, 1536).transpose(2, 0, 1, 3).astype(NPF8))
    wproj_p = np.ascontiguousarray(
        proj_w.T.reshape(2, 2, P, 512).transpose(2, 0, 1, 3).astype(NPF8))
    gnw_p = np.ascontiguousarray(gn_w.reshape(CT, P).T)
    gnb_p = np.ascontiguousarray(gn_b.reshape(CT, P).T)
    qkvb_p = np.ascontiguousarray(qkv_b[:1024].reshape(8, P).T)
    # fold the V bias through the projection: proj(att + vb) = proj(att) + W@vb
    projb_f = proj_b + proj_w @ qkv_b[1024:]
    projb_p = np.ascontiguousarray(projb_f.astype(np.float32).reshape(CT, P).T)
    sel = np.zeros((P, P), dtype=NPBF16)
    for p in range(P):
        sel[p, p // 16] = 1.0
    selt = np.ascontiguousarray(sel.T)

    shared = {
        "wqkv": wqkv_p, "wproj": wproj_p, "gnw": gnw_p, "gnb": gnb_p,
        "qkvb": qkvb_p, "projb": projb_p, "sel": sel, "selt": selt,
    }
    in_maps = []
    for i in range(N_CORES):
        m = dict(shared)
        xi = np.ascontiguousarray(x[i].reshape(512, S))
        m["x"] = xi
        m["xb"] = np.ascontiguousarray(xi.astype(NPBF16))
        in_maps.append(m)
    return in_maps


def run(inputs, trace=False, tmpdir=None):
    nc = _build()
    in_maps = _prep_inputs(inputs)
    res = run_bass_kernel_spmd(
        nc, in_maps, core_ids=list(range(N_CORES)), trace=trace, tmpdir=tmpdir
    )
    out = np.stack([res.results[i]["out"] for i in range(N_CORES)])
    return out.reshape(N_CORES, 512, 32, 32), res


def kernel(**inputs):
    out, _ = run(inputs, trace=False)
    return out


# revision 19
# speedup vs baseline: 1.1340x; 1.1340x over previous
"""AttentionBlock (GroupNorm + 8-head self-attention + proj + residual) on 8 TRN2 cores.

Sharding: data-parallel over batch (8 images -> 8 cores). Each core runs the full
block for one [512, 1024] image.

v2 design (vs the bf16 baseline):
  - All four matmul groups (QKV, scores, AV for heads 0-3, proj) run fp8(e4m3)
    DoubleRow (0.5 cyc/row). fp8 quantization is safe: the attention path is
    ~25x diluted vs the fp32 residual.
  - Scores per head: contraction d=64 at partition base (h%2)*64; DoubleRow
    plane 1 of lhsT points at a zeroed slot of the K tile, and the rhs reuses
    the same Q slab via a stride-0 plane (0 * Q = 0).
  - Softmax exp (64 [128,1024] psum tiles) is the elementwise bottleneck and
    PSUM is reachable only by ACT and DVE (GPSIMD/Pool is SBUF-only), so exp
    splits: ACT runs true exp (heads 0-3 -> fp8; heads 4-5 + part of 6 -> bf16
    written through a u16 bitcast), DVE runs the Schraudolph bit-hack
    (u16 = round(s*EA+EB) == bf16 bits of exp(s/8)) for head 7 + rest of 6.
    AV for heads 4-7 consumes the u16 tiles bitcast to bf16 (plain matmul).
  - AV stationary is [v | ones] (even heads) / [ones | v] (odd), so psum rows
    carry att on the head's own 64 partitions and 64 replicated softmax
    denominators Z on the other half: normalize = reciprocal_approx_fast +
    one psum-eviction multiply; no partition broadcast, no DMA staging.
  - qkv_b's v-part is folded into proj_b on the host (proj_w @ vb), removing
    the V bias tensor; V evictions are plain ACT Identity converts.
  - GroupNorm stats run on a bf16 copy of x (fast DMA); fp32 x streams in
    behind the weights and is only read by the final residual.
"""
import sys

sys.path.insert(0, "/opt/trn_rl_repo")

import numpy as np
import ml_dtypes

import concourse.bass as bass
import concourse.bacc as bacc
import concourse.tile as tile
from concourse import mybir
from concourse.bass_utils import run_bass_kernel_spmd

F32 = mybir.dt.float32
I32 = mybir.dt.int32
BF16 = mybir.dt.bfloat16
F8 = mybir.dt.float8e4
U16 = mybir.dt.uint16
AF = mybir.ActivationFunctionType
OP = mybir.AluOpType
DR = mybir.MatmulPerfMode.DoubleRow
NPBF16 = ml_dtypes.bfloat16
NPF8 = ml_dtypes.float8_e4m3

P = 128
S = 1024
CT = 4
HEADS = 8
D = 64
N_CORES = 8
EPS = 1e-5

# Schraudolph exp constants, folded with the softmax scale (1/8) and the
# >>16 that lands the bf16 bit pattern in a u16.
EA = (2.0**23 / float(np.log(2.0))) * 0.125 / 65536.0
EB = (127 * 2**23 - 486411) / 65536.0
# magic-constant fast reciprocal (bitcast(MAGIC - bits(z)) ~ 1/z, ~5% max err)
MAGIC = 0x7EF31200

F8_HEADS = (0, 1, 2, 3)    # ACT exp -> fp8 pt, DoubleRow AV
B16_HEADS = (4, 5, 6, 7)   # u16 pt (bf16 bits), plain bf16 AV
# exp engine per (head, chunk): ACT h0-5 + h6 c0-3; DVE-hack h6 c4-7, h7.
ACT_EXP = {(h, c) for h in range(6) for c in range(8)} | {(6, c) for c in range(4)}


def _emit(nc, tc, ctx):
    io = {
        "xb": nc.dram_tensor("xb", [512, S], BF16, kind="ExternalInput"),
        "x": nc.dram_tensor("x", [512, S], F32, kind="ExternalInput"),
        "wqkv": nc.dram_tensor("wqkv", [P, 2, 2, 1536], F8, kind="ExternalInput"),
        "wproj": nc.dram_tensor("wproj", [P, 2, 2, 512], F8, kind="ExternalInput"),
        "gnw": nc.dram_tensor("gnw", [P, CT], F32, kind="ExternalInput"),
        "gnb": nc.dram_tensor("gnb", [P, CT], F32, kind="ExternalInput"),
        "qkvb": nc.dram_tensor("qkvb", [P, 8], F32, kind="ExternalInput"),
        "projb": nc.dram_tensor("projb", [P, CT], F32, kind="ExternalInput"),
        "sel": nc.dram_tensor("sel", [P, P], BF16, kind="ExternalInput"),
        "selt": nc.dram_tensor("selt", [P, P], BF16, kind="ExternalInput"),
        "out": nc.dram_tensor("out", [512, S], F32, kind="ExternalOutput"),
    }
    _emit_io(nc, tc, ctx, io)


def _emit_io(nc, tc, ctx, io):
    consts = ctx.enter_context(tc.tile_pool(name="consts", bufs=1))
    big = ctx.enter_context(tc.tile_pool(name="big", bufs=1))
    small = ctx.enter_context(tc.tile_pool(name="small", bufs=2))
    ptf8p = ctx.enter_context(tc.tile_pool(name="ptf8", bufs=4))
    ptu16p = ctx.enter_context(tc.tile_pool(name="ptu16", bufs=3))
    zbp = ctx.enter_context(tc.tile_pool(name="zbp", bufs=2))
    outp = ctx.enter_context(tc.tile_pool(name="outp", bufs=3))
    ps = ctx.enter_context(tc.tile_pool(name="ps", bufs=4, space="PSUM"))

    # ---- input DMAs (bf16 x first: GroupNorm needs it immediately) ----
    xbv = io["xb"][:, :].rearrange("(j p) s -> p j s", p=P)
    xb = big.tile([P, CT, S], BF16)
    for j in range(CT):
        nc.sync.dma_start(out=xb[:, j, :], in_=xbv[:, j, :])
    wqkv = consts.tile([P, 2, 2, 1536], F8)
    nc.sync.dma_start(out=wqkv, in_=io["wqkv"][:, :, :, :])
    gnw = consts.tile([P, CT], F32)
    nc.sync.dma_start(out=gnw, in_=io["gnw"][:, :])
    gnb = consts.tile([P, CT], F32)
    nc.sync.dma_start(out=gnb, in_=io["gnb"][:, :])
    sel = consts.tile([P, P], BF16)
    nc.sync.dma_start(out=sel, in_=io["sel"][:, :])
    selt = consts.tile([P, P], BF16)
    nc.sync.dma_start(out=selt, in_=io["selt"][:, :])
    qkvb = consts.tile([P, 8], F32)
    nc.sync.dma_start(out=qkvb, in_=io["qkvb"][:, :])
    projb = consts.tile([P, CT], F32)
    nc.sync.dma_start(out=projb, in_=io["projb"][:, :])
    x_all = big.tile([P, CT, S], F32)
    xv = io["x"][:, :].rearrange("(j p) s -> p j s", p=P)
    for j in range(CT):
        nc.sync.dma_start(out=x_all[:, j, :], in_=xv[:, j, :])
    wproj = consts.tile([P, 2, 2, 512], F8)
    nc.sync.dma_start(out=wproj, in_=io["wproj"][:, :, :, :])

    eps_t = consts.tile([P, 1], F32)
    nc.vector.memset(eps_t, EPS)
    zeros8 = consts.tile([P, 8], F32)
    nc.vector.memset(zeros8, 0.0)

    h = big.tile([P, CT, S], F8)
    qf8 = big.tile([P, CT, S], F8)
    # K storage: [p, head, chunk, 128]. Head h occupies partitions
    # (h%2)*64..+64; the other 64 rows stay zero so the scores matmul can
    # contract over the full 128 partitions of the stacked Q pair tile.
    kf8 = big.tile([P, HEADS, 8, P], F8)
    for hh in range(HEADS):
        zb_base = 0 if hh % 2 == 1 else D
        nc.gpsimd.memset(kf8[zb_base:zb_base + D, hh, :, :], 0.0)
    # V stationary [v | ones] (even heads) / [ones | v] (odd): psum rows get
    # att on the head's own half and 64 replicated Z rows on the other half.
    vtf8 = big.tile([P, 8, len(F8_HEADS), P], F8)
    vtb16 = big.tile([P, 8, len(B16_HEADS), P], BF16)
    for i, hh in enumerate(F8_HEADS):
        off = D if hh % 2 == 0 else 0
        nc.gpsimd.memset(vtf8[:, :, i, off:off + D], 1.0)
    for i, hh in enumerate(B16_HEADS):
        off = D if hh % 2 == 0 else 0
        nc.gpsimd.memset(vtb16[:, :, i, off:off + D], 1.0)
    att = big.tile([P, CT, S], F8)

    # ---- GroupNorm statistics (DVE, on bf16 x) ----
    mv = small.tile([P, CT, 2], F32)
    for j in range(CT):
        stats = small.tile([P, 2, 6], F32, tag="st", name=f"st{j}")
        for sg in range(2):
            nc.vector.bn_stats(out=stats[:, sg, :], in_=xb[:, j, sg * 512:(sg + 1) * 512])
        nc.vector.bn_aggr(out=mv[:, j, :], in_=stats[:, :, :])
    means = mv[:, :, 0]
    vars_ = mv[:, :, 1]
    stats2 = small.tile([P, 8], F32)
    nc.vector.tensor_copy(out=stats2[:, 0:4], in_=means)
    nc.vector.tensor_mul(out=stats2[:, 4:8], in0=means, in1=means)
    nc.vector.tensor_add(out=stats2[:, 4:8], in0=stats2[:, 4:8], in1=vars_)
    statsr = small.tile([P, 8], BF16)
    nc.vector.tensor_copy(out=statsr, in_=stats2)

    psum_g = ps.tile([P, 8], F32, tag="mm")
    nc.tensor.matmul(psum_g[:, :], lhsT=sel[:, :], rhs=statsr[:, :], start=True, stop=True)

    tmv = small.tile([P, 8], F32)
    nc.vector.tensor_scalar_mul(out=tmv[0:8, :], in0=psum_g[0:8, :], scalar1=1.0 / 16.0)
    gm = tmv[0:8, 0:4]
    gm2 = tmv[0:8, 4:8]
    var_t = small.tile([P, 4], F32)
    nc.vector.tensor_mul(out=var_t[0:8, :], in0=gm, in1=gm)
    nc.vector.tensor_sub(out=var_t[0:8, :], in0=gm2, in1=var_t[0:8, :])
    nc.scalar.activation(out=var_t[0:8, :], in_=var_t[0:8, :], func=AF.Sqrt, bias=eps_t[0:8, :], scale=1.0)
    a_t = small.tile([P, 4], F32)
    nc.vector.reciprocal(out=a_t[0:8, :], in_=var_t[0:8, :])
    b_t = small.tile([P, 4], F32)
    nc.vector.tensor_mul(out=b_t[0:8, :], in0=gm, in1=a_t[0:8, :])
    abr = small.tile([P, 8], BF16)
    nc.vector.tensor_copy(out=abr, in_=zeros8)
    nc.vector.tensor_copy(out=abr[0:8, 0:4], in_=a_t[0:8, :])
    nc.vector.tensor_scalar(out=abr[0:8, 4:8], in0=b_t[0:8, :], scalar1=-1.0, scalar2=None, op0=OP.mult)

    psum_ab = ps.tile([P, 8], F32, tag="mm")
    nc.tensor.matmul(psum_ab[:, :], lhsT=selt[:, :], rhs=abr[:, :], start=True, stop=True)

    scb = small.tile([P, CT, 2], F32)
    for j in range(CT):
        nc.vector.tensor_mul(out=scb[:, j, 0:1], in0=psum_ab[:, j:j + 1], in1=gnw[:, j:j + 1])
        nc.vector.tensor_mul(out=scb[:, j, 1:2], in0=psum_ab[:, 4 + j:5 + j], in1=gnw[:, j:j + 1])
        nc.vector.tensor_add(out=scb[:, j, 1:2], in0=scb[:, j, 1:2], in1=gnb[:, j:j + 1])
        # GN apply: h = xb * a + b -> fp8. Chunks 0-1 on ACT (fast QKV
        # start), 2-3 on Pool (the only psum-free engine).
        if j < 2:
            nc.scalar.activation(out=h[:, j, :], in_=xb[:, j, :], func=AF.Identity,
                                 bias=scb[:, j, 1:2], scale=scb[:, j, 0:1])
        else:
            nc.gpsimd.tensor_scalar(out=h[:, j, :], in0=xb[:, j, :],
                                    scalar1=scb[:, j, 0:1], scalar2=scb[:, j, 1:2],
                                    op0=OP.mult, op1=OP.add)

    # ---- QKV emitters ----
    def emit_k(ko):
        pk = ps.tile([P, S], F32, tag="mm", name=f"pk{ko}")
        oc = (4 + ko) * P
        for pair in range(2):
            for nh in range(2):
                nc.tensor.matmul(
                    pk[:, nh * 512:(nh + 1) * 512],
                    lhsT=wqkv[:, pair, :, oc:oc + P],
                    rhs=h[:, 2 * pair:2 * pair + 2, nh * 512:(nh + 1) * 512],
                    start=(pair == 0), stop=(pair == 1), perf_mode=DR,
                )
        h_even, h_odd = 2 * ko, 2 * ko + 1
        if ko in (0, 3):
            nc.vector.tensor_scalar(
                out=kf8[0:D, h_even, :, :],
                in0=pk[0:D, :].rearrange("p (c n) -> p c n", c=8),
                scalar1=qkvb[0:D, 4 + ko:5 + ko], scalar2=None, op0=OP.add)
            nc.vector.tensor_scalar(
                out=kf8[D:P, h_odd, :, :],
                in0=pk[D:P, :].rearrange("p (c n) -> p c n", c=8),
                scalar1=qkvb[D:P, 4 + ko:5 + ko], scalar2=None, op0=OP.add)
        else:
            nc.scalar.activation(
                out=kf8[0:D, h_even, :, :],
                in_=pk[0:D, :].rearrange("p (c n) -> p c n", c=8),
                func=AF.Identity, bias=qkvb[0:D, 4 + ko:5 + ko], scale=1.0)
            nc.scalar.activation(
                out=kf8[D:P, h_odd, :, :],
                in_=pk[D:P, :].rearrange("p (c n) -> p c n", c=8),
                func=AF.Identity, bias=qkvb[D:P, 4 + ko:5 + ko], scale=1.0)

    def emit_q(pc):
        pq = ps.tile([P, S], F32, tag="mm", name=f"pq{pc}")
        for pair in range(2):
            for nh in range(2):
                nc.tensor.matmul(
                    pq[:, nh * 512:(nh + 1) * 512],
                    lhsT=wqkv[:, pair, :, pc * P:(pc + 1) * P],
                    rhs=h[:, 2 * pair:2 * pair + 2, nh * 512:(nh + 1) * 512],
                    start=(pair == 0), stop=(pair == 1), perf_mode=DR,
                )
        nc.scalar.activation(out=qf8[:, pc, :], in_=pq[:, :], func=AF.Identity,
                             bias=qkvb[:, pc:pc + 1], scale=1.0)

    def emit_v(si):
        pv = ps.tile([P, 512], F32, tag="mm", name=f"pv{si}")
        for pair in range(2):
            nc.tensor.matmul(
                pv[:, :],
                lhsT=h[:, 2 * pair:2 * pair + 2, si * P:(si + 1) * P],
                rhs=wqkv[:, pair, :, 1024:1536],
                start=(pair == 0), stop=(pair == 1), perf_mode=DR,
            )
        src = pv[:, :].rearrange("p (hh d) -> p hh d", hh=8)
        # per (dtype, parity) group; stride-2 head slices keep APs regular
        for dst, heads, par in ((vtf8, F8_HEADS, 0), (vtf8, F8_HEADS, 1),
                                (vtb16, B16_HEADS, 0), (vtb16, B16_HEADS, 1)):
            idxs = [i for i, hh in enumerate(heads) if hh % 2 == par]
            gh = [heads[i] for i in idxs]
            off = 0 if par == 0 else D
            st = (idxs[1] - idxs[0]) if len(idxs) > 1 else 1
            d0 = dst[:, si, idxs[0]:idxs[-1] + 1:st, off:off + D]
            s0 = src[:, gh[0]:gh[-1] + 1:2, :] if len(gh) > 1 else src[:, gh[0]:gh[0] + 1, :]
            nc.vector.tensor_copy(out=d0, in_=s0)

    def emit_score_chunk(hh, c, pt):
        psc = ps.tile([P, S], F32, tag="mm", name=f"sc{hh}_{c}")
        for ah in range(2):
            nc.tensor.matmul(
                psc[:, ah * 512:(ah + 1) * 512],
                lhsT=kf8[:, hh, c, :],
                rhs=qf8[:, hh // 2, ah * 512:(ah + 1) * 512],
                start=True, stop=True,
            )
        if hh in F8_HEADS:
            nc.scalar.activation(out=pt[:, c, :], in_=psc[:, :], func=AF.Exp, scale=0.125)
        elif (hh, c) in ACT_EXP:
            nc.scalar.activation(out=pt[:, c, :].bitcast(BF16), in_=psc[:, :],
                                 func=AF.Exp, scale=0.125)
        else:
            nc.vector.tensor_scalar(out=pt[:, c, :], in0=psc[:, :],
                                    scalar1=EA, scalar2=EB, op0=OP.mult, op1=OP.add)

    def new_pt(hh):
        if hh in F8_HEADS:
            return ptf8p.tile([P, 8, S], F8, tag="pt", name=f"ptf8_{hh}")
        return ptu16p.tile([P, 8, S], U16, tag="pt", name=f"ptu16_{hh}")

    def av_steps(hh):
        return 4 if hh in F8_HEADS else 8

    def emit_av_mms(hh, pt, pav, step):
        last = av_steps(hh) - 1
        if hh in F8_HEADS:
            vt = vtf8[:, :, F8_HEADS.index(hh), :]
            for ah in range(2):
                nc.tensor.matmul(
                    pav[:, ah * 512:(ah + 1) * 512],
                    lhsT=vt[:, 2 * step:2 * step + 2, :],
                    rhs=pt[:, 2 * step:2 * step + 2, ah * 512:(ah + 1) * 512],
                    start=(step == 0), stop=(step == last), perf_mode=DR,
                    skip_group_check=True,
                )
        else:
            vt = vtb16[:, :, B16_HEADS.index(hh), :]
            for ah in range(2):
                nc.tensor.matmul(
                    pav[:, ah * 512:(ah + 1) * 512],
                    lhsT=vt[:, step, :],
                    rhs=pt[:, step, ah * 512:(ah + 1) * 512].bitcast(BF16),
                    start=(step == 0), stop=(step == last),
                    skip_group_check=True,
                )

    def emit_av(hh, pt, pav=None):
        if pav is None:
            pav = ps.tile([P, S], F32, tag="mm", name=f"av{hh}")
            for step in range(av_steps(hh)):
                emit_av_mms(hh, pt, pav, step)
        # normalize; even head: att rows 0:64, Z rows 64:128 (odd: flipped).
        # 1/Z via the magic-constant int trick: DVE microcoded reciprocal ops
        # read garbage from PSUM on real HW, tensor_scalar int ops don't.
        emit_av_norm(hh, pav)

    def emit_av_norm(hh, pav):
        zbinv = zbp.tile([P, S], I32, tag="zb", name=f"zb{hh}")
        if hh % 2 == 0:
            nc.vector.tensor_scalar(out=zbinv[D:P, :], in0=pav[D:P, :].bitcast(I32),
                                    scalar1=-1, scalar2=MAGIC, op0=OP.mult, op1=OP.add)
            nc.vector.tensor_mul(out=att[0:D, hh // 2, :], in0=pav[0:D, :],
                                 in1=zbinv[D:P, :].bitcast(F32))
        else:
            nc.vector.tensor_scalar(out=zbinv[0:D, :], in0=pav[0:D, :].bitcast(I32),
                                    scalar1=-1, scalar2=MAGIC, op0=OP.mult, op1=OP.add)
            nc.vector.tensor_mul(out=att[D:P, hh // 2, :], in0=pav[D:P, :],
                                 in1=zbinv[0:D, :].bitcast(F32))

    # ---- proj + bias + residual ----
    out_view = io["out"][:, :].rearrange("(j p) s -> p j s", p=P)

    def proj_mms(pp, oi, pair):
        for sh in range(2):
            nc.tensor.matmul(
                pp[:, sh * 512:(sh + 1) * 512],
                lhsT=wproj[:, pair, :, oi * P:(oi + 1) * P],
                rhs=att[:, 2 * pair:2 * pair + 2, sh * 512:(sh + 1) * 512],
                start=(pair == 0), stop=(pair == 1), perf_mode=DR,
                skip_group_check=True,
            )

    def proj_evict(pp, oi):
        ot = outp.tile([P, S], F32, tag="o")
        nc.vector.scalar_tensor_tensor(out=ot, in0=pp[:, :], scalar=projb[:, oi:oi + 1],
                                       in1=x_all[:, oi, :], op0=OP.add, op1=OP.add)
        nc.sync.dma_start(out=out_view[:, oi, :], in_=ot)


    # ---- emission schedule ----
    # PE: all K/Q | sc0+sc6 | sc1+sc7 | V | sc2 | av0 av6 | sc3 | av1 av7 |
    #     sc4 | av2 | sc5 | av3 av4 av5 | proj.  ACT exp streams h0..h5;
    # DVE hacks h6, h7 early, h5 tail; Pool = memsets + GN applies.
    pts = {hh: None for hh in range(8)}
    emit_k(0)
    emit_q(0)
    emit_k(3)
    emit_q(3)
    emit_k(1)
    emit_q(1)
    emit_k(2)
    emit_q(2)
    pts[0] = new_pt(0)
    pts[6] = new_pt(6)
    for c in range(8):
        emit_score_chunk(0, c, pts[0])
        emit_score_chunk(6, c, pts[6])
    pts[1] = new_pt(1)
    pts[7] = new_pt(7)
    for c in range(8):
        emit_score_chunk(1, c, pts[1])
        emit_score_chunk(7, c, pts[7])
    for si in range(8):
        emit_v(si)
    pts[2] = new_pt(2)
    for c in range(8):
        emit_score_chunk(2, c, pts[2])
    emit_av(0, pts[0])
    emit_av(6, pts[6])
    pts[3] = new_pt(3)
    for c in range(8):
        emit_score_chunk(3, c, pts[3])
    emit_av(1, pts[1])
    emit_av(7, pts[7])
    pts[4] = new_pt(4)
    for c in range(8):
        emit_score_chunk(4, c, pts[4])
    emit_av(2, pts[2])
    pts[5] = new_pt(5)
    for c in range(8):
        emit_score_chunk(5, c, pts[5])
    emit_av(3, pts[3])
    emit_av(4, pts[4])
    emit_av(5, pts[5])
    for oi in range(CT):
        pp = ps.tile([P, S], F32, tag="mm", name=f"pp{oi}")
        proj_mms(pp, oi, 0)
        proj_mms(pp, oi, 1)
        proj_evict(pp, oi)


_NC_CACHE = None


def _build():
    global _NC_CACHE
    if _NC_CACHE is None:
        from contextlib import ExitStack

        nc = bacc.Bacc()
        with tile.TileContext(nc) as tc:
            with ExitStack() as ctx:
                _emit(nc, tc, ctx)
        nc.finalize()
        _NC_CACHE = nc
    return _NC_CACHE


def _prep_inputs(inputs):
    x = np.ascontiguousarray(np.asarray(inputs["x"], dtype=np.float32))  # [8,512,32,32]
    gn_w = np.asarray(inputs["gn_w"], dtype=np.float32)
    gn_b = np.asarray(inputs["gn_b"], dtype=np.float32)
    qkv_w = np.asarray(inputs["qkv_w"], dtype=np.float32)
    qkv_b = np.asarray(inputs["qkv_b"], dtype=np.float32)
    proj_w = np.asarray(inputs["proj_w"], dtype=np.float32)
    proj_b = np.asarray(inputs["proj_b"], dtype=np.float32)

    # [p, pair, plane, o] with c = (2*pair+plane)*128 + p
    wqkv_p = np.ascontiguousarray(
        qkv_w.T.reshape(2, 2, P, 1536).transpose(2, 0, 1, 3).astype(NPF8))
    wproj_p = np.ascontiguousarray(
        proj_w.T.reshape(2, 2, P, 512).transpose(2, 0, 1, 3).astype(NPF8))
    gnw_p = np.ascontiguousarray(gn_w.reshape(CT, P).T)
    gnb_p = np.ascontiguousarray(gn_b.reshape(CT, P).T)
    qkvb_p = np.ascontiguousarray(qkv_b[:1024].reshape(8, P).T)
    # fold the V bias through the projection: proj(att + vb) = proj(att) + W@vb
    projb_f = proj_b + proj_w @ qkv_b[1024:]
    projb_p = np.ascontiguousarray(projb_f.astype(np.float32).reshape(CT, P).T)
    sel = np.zeros((P, P), dtype=NPBF16)
    for p in range(P):
        sel[p, p // 16] = 1.0
    selt = np.ascontiguousarray(sel.T)

    shared = {
        "wqkv": wqkv_p, "wproj": wproj_p, "gnw": gnw_p, "gnb": gnb_p,
        "qkvb": qkvb_p, "projb": projb_p, "sel": sel, "selt": selt,
    }
    in_maps = []
    for i in range(N_CORES):
        m = dict(shared)
        xi = np.ascontiguousarray(x[i].reshape(512, S))
        m["x"] = xi
        m["xb"] = np.ascontiguousarray(xi.astype(NPBF16))
        in_maps.append(m)
    return in_maps


def run(inputs, trace=False, tmpdir=None):
    nc = _build()
    in_maps = _prep_inputs(inputs)
    res = run_bass_kernel_spmd(
        nc, in_maps, core_ids=list(range(N_CORES)), trace=trace, tmpdir=tmpdir
    )
    out = np.stack([res.results[i]["out"] for i in range(N_CORES)])
    return out.reshape(N_CORES, 512, 32, 32), res


def kernel(**inputs):
    out, _ = run(inputs, trace=False)
    return out
